# revision 16
# baseline (speedup 1.0000x reference)
"""Trainium2 Bass kernel for nn_DecoderLayer (self-attn + cross-attn + FFN).

Sharding: 8 cores = 4 batch elements x 2 query-token halves (no collectives).
Each core runs the full layer for 512 query tokens of one batch element,
computing full-sequence K/V locally (feature-major layout, fp16 matmuls,
fp32 accumulation / LN / softmax plumbing).

Layout convention on chip: activations are FEATURE-major [feature, token]
so every matmul contracts over the partition dim with zero transposes,
except attention-V which is produced token-major directly and probs which
are produced keys-major; softmax denominators come free via an appended
ones-column on V; per-query normalization is a per-partition scalar.
"""

import sys

for _p in ("/opt/trn_rl_repo",):
    if _p not in sys.path:
        sys.path.insert(0, _p)

import numpy as np

import concourse.bass as bass
from concourse import bacc
import concourse.mybir as mybir
import concourse.tile as tile
from concourse.masks import make_identity

T, S, B, D, H, DH, DI = 1024, 1024, 4, 1024, 16, 64, 4096
P = 128
NQ = T // 2          # queries per core
DC = D // P          # 8  d chunks
KC = T // P          # 8  key chunks (self)
EC = S // P          # 8  key chunks (cross)
QC = NQ // P         # 4  query chunks
HC = (H * DH) // P   # 8  head-feature chunks
FIC = DI // P        # 32 ffn inner chunks
SCALE = 1.0 / (DH ** 0.5)
EPS = 1e-5

F16 = mybir.dt.float16
F32 = mybir.dt.float32
AF = mybir.ActivationFunctionType
OP = mybir.AluOpType

_CACHE = {}


def _build(debug=False):
    nc = bacc.Bacc()

    dx = nc.dram_tensor("x_fm", [D, T], F16, kind="ExternalInput")
    dxq = nc.dram_tensor("xq_fm", [D, NQ], F16, kind="ExternalInput")
    denc = nc.dram_tensor("enc_fm", [D, S], F16, kind="ExternalInput")
    dmask = nc.dram_tensor("maskT", [T, NQ], F16, kind="ExternalInput")
    demask = nc.dram_tensor("emask", [P, EC], F32, kind="ExternalInput")
    dw = {}
    for nm, shp in [
        ("wq1", [D, H * DH]), ("wk1", [D, H * DH]), ("wv1", [D, H * DH]),
        ("wo1", [H * DH, D]),
        ("wq2", [D, H * DH]), ("wk2", [D, H * DH]), ("wv2", [D, H * DH]),
        ("wo2", [H * DH, D]),
        ("wff1", [D, DI]), ("wff2", [DI, D]),
    ]:
        dw[nm] = nc.dram_tensor(nm, shp, F16, kind="ExternalInput")
    dln = {}
    for nm, w in [("g1", DC), ("b1", DC), ("g2", DC), ("b2", DC),
                  ("g3", DC), ("b3", DC), ("bf1", FIC), ("bf2", DC)]:
        dln[nm] = nc.dram_tensor(nm, [P, w], F32, kind="ExternalInput")
    dout = nc.dram_tensor("out_fm", [D, NQ], F32, kind="ExternalOutput")

    with tile.TileContext(nc) as tc:
        with tc.tile_pool(name="sb", bufs=1) as sb, \
             tc.tile_pool(name="pp", bufs=2, space="PSUM") as pp:

            # ---------------- constants ----------------
            ident = sb.tile([P, P], F16, tag="ident", name="ident")
            make_identity(nc, ident)
            ones_bc = sb.tile([1, P], F16, tag="onesbc", name="onesbc")
            nc.vector.memset(ones_bc, 1.0)
            oneD = sb.tile([P, 1], F16, tag="oneD", name="oneD")
            nc.vector.memset(oneD, 1.0 / D)
            epsT = sb.tile([1, 1], F32, tag="epsT", name="epsT")
            nc.vector.memset(epsT, EPS)
            ln = {}
            for nm in dln:
                w = FIC if nm == "bf1" else DC
                ln[nm] = sb.tile([P, w], F32, tag=nm, name=nm)
                nc.sync.dma_start(ln[nm], dln[nm][:, :])
            emask_sb = sb.tile([P, EC], F32, tag="emask", name="emask")
            nc.sync.dma_start(emask_sb, demask[:, :])
            mask_sb = []
            dmask_r = dmask.rearrange("(kc p) q -> p kc q", p=P)
            for kc in range(KC):
                mt = sb.tile([P, NQ], F16, tag=f"m{kc}", name=f"m{kc}")
                nc.sync.dma_start(mt, dmask_r[:, kc, :])
                mask_sb.append(mt)

            def fam(prefix, n, shape, dtype):
                return [sb.tile(shape, dtype, tag=f"{prefix}{i}", name=f"{prefix}{i}")
                        for i in range(n)]

            def dbg(name, tiles):
                if not debug:
                    return
                w = tiles[0].shape[-1] if len(tiles[0].shape) == 2 else (
                    tiles[0].shape[1] * tiles[0].shape[2])
                dt_ = nc.dram_tensor(name, [len(tiles) * P, w], tiles[0].dtype,
                                     kind="ExternalOutput")
                r = dt_.rearrange("(c p) t -> p c t", p=P)
                for i, t_ in enumerate(tiles):
                    if len(t_.shape) == 3:
                        t_ = t_.rearrange("p a b -> p (a b)")
                    nc.sync.dma_start(r[:, i, :], t_)

            # ---------------- phase A: load x, LN1 (in-place x -> c) -------
            e_t = fam("e", DC, [P, T], F16)          # x, then c, later enc
            dx_r = dx.rearrange("(dc p) t -> p dc t", p=P)
            for dc in range(DC):
                nc.sync.dma_start(e_t[dc], dx_r[:, dc, :])
            xq_t = fam("q", DC, [P, NQ], F16)        # xq (q-proj rhs + residual)
            dxq_r = dxq.rearrange("(dc p) t -> p dc t", p=P)
            for dc in range(DC):
                nc.sync.dma_start(xq_t[dc], dxq_r[:, dc, :])

            def ln_fm(src, ntok, g, b, out16=None, out32=None, ones=None,
                      dbgname=None):
                """Feature-major layernorm over partition (feature) dim.

                src: list of DC tiles [P, ntok] (f16 or f32).
                Writes out32 (f32, optional) and/or out16 (f16) tiles.
                """
                src16 = src[0].dtype == F16
                for j in range(ntok // 512):
                    js = slice(j * 512, (j + 1) * 512)
                    ms = pp.tile([1, 512], F32, tag="av", name="av")
                    qs = pp.tile([1, 512], F32, tag="av", name="av")
                    for dc in range(DC):
                        xs = src[dc][:, js]
                        if src16:
                            xh = xs
                        else:
                            xh = sb.tile([P, 512], F16, tag="lxh", name="lxh", bufs=2)
                            nc.vector.tensor_copy(xh, xs)
                        sq = sb.tile([P, 512], F16, tag="lsq", name="lsq", bufs=2)
                        nc.vector.tensor_mul(sq, xh, xh)
                        nc.tensor.matmul(ms, lhsT=ones, rhs=xh,
                                         start=(dc == 0), stop=(dc == DC - 1))
                        nc.tensor.matmul(qs, lhsT=ones, rhs=sq,
                                         start=(dc == 0), stop=(dc == DC - 1))
                    mean = sb.tile([1, 512], F32, tag="stm", name="stm", bufs=1)
                    nc.vector.tensor_copy(mean, ms)
                    m2 = sb.tile([1, 512], F32, tag="st2", name="st2", bufs=1)
                    nc.vector.tensor_mul(m2, mean, mean)
                    var = sb.tile([1, 512], F32, tag="stv", name="stv", bufs=1)
                    nc.vector.tensor_sub(var, qs, m2)
                    std = sb.tile([1, 512], F32, tag="sts", name="sts", bufs=1)
                    nc.scalar.activation(std, var, AF.Sqrt, bias=epsT)
                    rstd = sb.tile([1, 512], F32, tag="str", name="str", bufs=1)
                    nc.vector.reciprocal(rstd, std)
                    if debug and dbgname is not None:
                        ex2 = sb.tile([1, 512], F32, tag="dbgx", name="dbgx",
                                      bufs=2)
                        nc.vector.tensor_copy(ex2, qs)
                        ddt = nc.dram_tensor(f"{dbgname}_{j}", [1, 1536], F32,
                                             kind="ExternalOutput")
                        nc.sync.dma_start(ddt[:, 0:512], mean)
                        nc.sync.dma_start(ddt[:, 512:1024], rstd)
                        nc.sync.dma_start(ddt[:, 1024:1536], ex2)
                    sh = sb.tile([1, 1024], F16, tag="sth", name="sth", bufs=1)
                    nc.vector.tensor_copy(sh[:, 0:512], mean)
                    nc.vector.tensor_copy(sh[:, 512:1024], rstd)
                    bcm = pp.tile([P, 512], F32, tag="big", name="big", bufs=4)
                    nc.tensor.matmul(bcm, lhsT=ones_bc, rhs=sh[:, 0:512],
                                     start=True, stop=True)
                    bcr = pp.tile([P, 512], F32, tag="big", name="big", bufs=4)
                    nc.tensor.matmul(bcr, lhsT=ones_bc, rhs=sh[:, 512:1024],
                                     start=True, stop=True)
                    for dc in range(DC):
                        t = sb.tile([P, 512], F16, tag="lnt", name="lnt", bufs=2)
                        nc.vector.tensor_sub(t, src[dc][:, js], bcm)
                        nc.vector.tensor_mul(t, t, bcr)
                        gc, bc_ = g[:, dc:dc + 1], b[:, dc:dc + 1]
                        if out32 is not None:
                            nc.vector.tensor_scalar(
                                out32[dc][:, js], t, gc, bc_, OP.mult, OP.add)
                            if out16 is not None:
                                nc.vector.tensor_copy(out16[dc][:, js],
                                                      out32[dc][:, js])
                        else:
                            nc.vector.tensor_scalar(
                                out16[dc][:, js], t, gc, bc_, OP.mult, OP.add)


            # ---------------- generic column-block projection --------------
            def proj_cols(wd, rhs, n_fc, writer, rhs_w=NQ):
                wr = wd.rearrange("(dc p) f -> p dc f", p=P)
                for fc in range(n_fc):
                    wt = sb.tile([P, DC, P], F16, tag="wb", name="wb", bufs=3)
                    nc.sync.dma_start(wt, wr[:, :, fc * P:(fc + 1) * P])
                    for jn in range(rhs_w // 512):
                        js = slice(jn * 512, (jn + 1) * 512)
                        acc = pp.tile([P, 512], F32, tag="big", name="big", bufs=4)
                        for dc in range(DC):
                            nc.tensor.matmul(
                                acc, lhsT=wt[:, dc, :], rhs=rhs[dc][:, js],
                                start=(dc == 0), stop=(dc == DC - 1))
                        writer(fc, acc) if rhs_w == 512 else writer(fc, acc, js)

            def proj_v(wd, src, va):
                """token-major V projection with appended ones column."""
                wr = wd.rearrange("(dc p) f -> p dc f", p=P)
                for grp in range(KC // 2):
                    accs = [[pp.tile([P, 512], F32, tag="big", name="big", bufs=4)
                             for _ in range(2)] for _ in range(2)]
                    for dc in range(DC):
                        wvt = sb.tile([P, H * DH], F16, tag="wv", name="wv", bufs=2)
                        nc.sync.dma_start(wvt, wr[:, dc, :])
                        for i in range(2):
                            tc8 = grp * 2 + i
                            for jn in range(2):
                                js = slice(jn * 512, (jn + 1) * 512)
                                nc.tensor.matmul(
                                    accs[i][jn],
                                    lhsT=src[dc][:, tc8 * P:(tc8 + 1) * P],
                                    rhs=wvt[:, js],
                                    start=(dc == 0), stop=(dc == DC - 1))
                    for i in range(2):
                        tc8 = grp * 2 + i
                        for jn in range(2):
                            nc.vector.tensor_copy(
                                va[tc8][:, jn * (H // 2):(jn + 1) * (H // 2), 0:DH],
                                accs[i][jn].rearrange("p (h d) -> p h d", h=H // 2))
                        nc.gpsimd.memset(va[tc8][:, :, DH:DH + 1], 1.0)

            # ---------------- phase B: self-attn projections ----------------
            t_t = fam("t", HC, [P, NQ], F16)         # q1, later h2_h
            k_t = fam("k", HC, [P, T], F16)          # k1, later k2
            va_t = fam("va", KC, [P, H, DH + 1], F16)  # v1(+ones), later v2

            # q1 only needs xq — emit before LN1 so PE is busy during LN1
            proj_cols(dw["wq1"], xq_t, HC,
                      lambda fc, acc: nc.vector.tensor_copy(t_t[fc], acc))

            ln_fm(e_t, T, ln["g1"], ln["b1"], out16=e_t, ones=oneD)  # c in e_t
            dbg("dbg_c", e_t)
            proj_cols(dw["wk1"], e_t, HC,
                      lambda fc, acc, js: nc.vector.tensor_copy(k_t[fc][:, js], acc),
                      rhs_w=T)
            proj_v(dw["wv1"], e_t, va_t)
            dbg("dbg_q1", t_t)
            dbg("dbg_k1", k_t)
            dbg("dbg_va", va_t)

            # ---------------- attention ----------------
            def attention(qt, kt, va, vec, masked):
                # masked (self) attention: causal mask is ACCUMULATED into the
                # score psum by an identity matmul (PE), and exp runs straight
                # from psum per 128-query block so fully-masked blocks
                # (kc > qc + 4 for every core) are skipped entirely.
                nkc = KC if masked else EC

                def kc_limit(qc):
                    return min(nkc, qc + 5) if masked else nkc

                for h in range(H):
                    fch, row = h // 2, (h % 2) * DH
                    plist = []
                    for kc in range(nkc):
                        sp = pp.tile([P, 512], F32, tag="big", name="big", bufs=4)
                        nc.tensor.matmul(
                            sp, lhsT=kt[fch][row:row + DH, kc * P:(kc + 1) * P],
                            rhs=qt[fch][row:row + DH, :],
                            start=True, stop=not masked)
                        pt = sb.tile([P, 512], F16, tag="p", name="p", bufs=8)
                        if masked:
                            nc.tensor.matmul(sp, lhsT=ident, rhs=mask_sb[kc],
                                             start=False, stop=True)
                            q0 = max(0, kc - 4) * P   # first non-skipped block
                            nc.scalar.activation(pt[:, q0:], sp[:, q0:],
                                                 AF.Exp, scale=SCALE)
                        else:
                            nc.scalar.activation(pt, sp, AF.Exp,
                                                 bias=emask_sb[:, kc:kc + 1],
                                                 scale=SCALE)
                        plist.append(pt)
                    for qc in range(QC):
                        lim = kc_limit(qc)
                        av = pp.tile([P, DH + 1], F32, tag="av", name="av")
                        for kc in range(lim):
                            nc.tensor.matmul(
                                av, lhsT=plist[kc][:, qc * P:(qc + 1) * P],
                                rhs=va[kc][:, h, :],
                                start=(kc == 0), stop=(kc == lim - 1))
                        rc = sb.tile([P, 1], F32, tag="rc", name="rc", bufs=3)
                        nc.vector.reciprocal(rc, av[:, DH:DH + 1])
                        vt = sb.tile([P, DH], F16, tag="vt", name="vt", bufs=3)
                        nc.vector.tensor_scalar_mul(vt, av[:, 0:DH], rc)
                        tp = pp.tile([DH, P], F16, tag="tr", name="tr")
                        nc.tensor.transpose(tp, vt, ident)
                        nc.vector.tensor_copy(
                            vec[fch][row:row + DH, qc * P:(qc + 1) * P], tp)

            vec_t = fam("s", HC, [P, NQ], F16)       # vec1, later vec2, later h3
            attention(t_t, k_t, va_t, vec_t, masked=True)
            dbg("dbg_vec", vec_t)

            # ---------------- phase D: Wo1 + residual, LN2 -----------------
            r_t = fam("r", DC, [P, NQ], F32)         # out1 -> h2 (f32)
            proj_cols(dw["wo1"], vec_t, DC,
                      lambda fc, acc: nc.vector.tensor_add(r_t[fc], acc, xq_t[fc]))
            dbg("dbg_out1", r_t)

            # cross-attn K/V don't depend on LN2 — emit early so PE has work
            # while LN2's serial stats chain runs.
            e2_t = fam("e", DC, [P, S], F16)         # enc (reuse e family)
            denc_r = denc.rearrange("(dc p) t -> p dc t", p=P)
            for dc in range(DC):
                nc.sync.dma_start(e2_t[dc], denc_r[:, dc, :])
            k2_t = fam("k", HC, [P, S], F16)
            va2_t = fam("va", EC, [P, H, DH + 1], F16)
            proj_cols(dw["wk2"], e2_t, HC,
                      lambda fc, acc, js: nc.vector.tensor_copy(k2_t[fc][:, js], acc),
                      rhs_w=S)
            proj_v(dw["wv2"], e2_t, va2_t)

            h2h_t = fam("t", HC, [P, NQ], F16)       # reuse t family
            ln_fm(r_t, NQ, ln["g2"], ln["b2"], out16=h2h_t, out32=r_t, ones=oneD, dbgname="dbg_ln2")
            dbg("dbg_h2", r_t)

            # ---------------- phase E: cross-attention ----------------
            q2_t = fam("m", HC, [P, NQ], F16)        # reuse mask family
            proj_cols(dw["wq2"], h2h_t, HC,
                      lambda fc, acc: nc.vector.tensor_copy(q2_t[fc], acc))

            vec2_t = fam("s", HC, [P, NQ], F16)
            attention(q2_t, k2_t, va2_t, vec2_t, masked=False)
            dbg("dbg_vec2", vec2_t)

            w_t = fam("w", DC, [P, NQ], F32)         # out2
            proj_cols(dw["wo2"], vec2_t, DC,
                      lambda fc, acc: nc.vector.tensor_add(w_t[fc], acc, r_t[fc]))
            dbg("dbg_out2", w_t)

            # ---------------- phase F: LN3 + FFN ----------------
            h3_t = fam("s", DC, [P, NQ], F16)
            ln_fm(w_t, NQ, ln["g3"], ln["b3"], out16=h3_t, ones=oneD)
            dbg("dbg_h3", h3_t)

            g_t = fam("gg", FIC, [P, NQ], F16)
            proj_cols(dw["wff1"], h3_t, FIC,
                      lambda fc, acc: nc.scalar.activation(
                          g_t[fc], acc, AF.Gelu,
                          bias=ln["bf1"][:, fc:fc + 1], scale=1.0))

            dout_r = dout.rearrange("(dc p) q -> p dc q", p=P)
            w2r = dw["wff2"].rearrange("(fic p) f -> p fic f", p=P)
            for dc in range(DC):
                acc = pp.tile([P, 512], F32, tag="big", name="big", bufs=4)
                for half in range(2):
                    w2t = sb.tile([P, FIC // 2, P], F16, tag="wf2", name="wf2",
                                  bufs=2)
                    nc.sync.dma_start(
                        w2t, w2r[:, half * (FIC // 2):(half + 1) * (FIC // 2),
                                 dc * P:(dc + 1) * P])
                    for f in range(FIC // 2):
                        fic = half * (FIC // 2) + f
                        nc.tensor.matmul(acc, lhsT=w2t[:, f, :], rhs=g_t[fic],
                                         start=(fic == 0), stop=(fic == FIC - 1))
                fin = sb.tile([P, NQ], F32, tag=f"r{dc}", name=f"r{dc}")  # reuse r slot
                nc.vector.tensor_scalar_add(fin, acc, ln["bf2"][:, dc:dc + 1])
                nc.vector.tensor_add(fin, fin, w_t[dc])
                nc.sync.dma_start(dout_r[:, dc, :], fin)

    nc.compile()
    return nc


def get_nc(debug=False):
    key = ("nc", debug)
    if key not in _CACHE:
        _CACHE[key] = _build(debug=debug)
    return _CACHE[key]


def make_in_maps(dec_inp, enc_out, dec_mask, enc_mask,
                 W_q1, W_kv1, W_o1, g1, b1,
                 W_q2, W_kv2, W_o2, g2, b2,
                 W_ff1, b_ff1, W_ff2, b_ff2, g3, b3):
    f16 = np.float16
    f32 = np.float32

    def colmajor(v, w):  # [P*w] -> [P, w]
        return np.ascontiguousarray(np.asarray(v, f32).reshape(w, P).T)

    shared = {
        "wq1": np.asarray(W_q1, f16),
        "wk1": np.asarray(W_kv1[:, :H * DH], f16),
        "wv1": np.ascontiguousarray(np.asarray(W_kv1[:, H * DH:], f16)),
        "wo1": np.asarray(W_o1, f16),
        "wq2": np.asarray(W_q2, f16),
        "wk2": np.asarray(W_kv2[:, :H * DH], f16),
        "wv2": np.ascontiguousarray(np.asarray(W_kv2[:, H * DH:], f16)),
        "wo2": np.asarray(W_o2, f16),
        "wff1": np.asarray(W_ff1, f16),
        "wff2": np.asarray(W_ff2, f16),
        "g1": colmajor(g1, DC), "b1": colmajor(b1, DC),
        "g2": colmajor(g2, DC), "b2": colmajor(b2, DC),
        "g3": colmajor(g3, DC), "b3": colmajor(b3, DC),
        "bf1": colmajor(b_ff1, FIC), "bf2": colmajor(b_ff2, DC),
    }
    dec_inp = np.asarray(dec_inp, f32)
    enc_out = np.asarray(enc_out, f32)
    dec_mask = np.asarray(dec_mask)
    enc_mask = np.asarray(enc_mask)
    in_maps = []
    for core in range(8):
        b, th = divmod(core, 2)
        r0 = th * NQ
        x_fm = np.ascontiguousarray(dec_inp[:, b, :].T.astype(f16))
        xq_fm = np.ascontiguousarray(dec_inp[r0:r0 + NQ, b, :].T.astype(f16))
        enc_fm = np.ascontiguousarray(enc_out[:, b, :].T.astype(f16))
        mT = dec_mask[r0:r0 + NQ, :, b].T                  # [T, NQ] bool
        maskT = np.where(mT, f16(-60000.0), f16(0.0))
        emask = np.ascontiguousarray(
            np.where(enc_mask[:, b], -10000.0, 0.0).astype(f32).reshape(EC, P).T)
        in_maps.append(dict(shared, x_fm=x_fm, xq_fm=xq_fm, enc_fm=enc_fm,
                            maskT=maskT, emask=emask))
    return in_maps


def assemble(results):
    out = np.empty((T, B, D), np.float32)
    for core in range(8):
        b, th = divmod(core, 2)
        r0 = th * NQ
        out[r0:r0 + NQ, b, :] = results[core]["out_fm"].T
    return out


def kernel(**inputs):
    from concourse.bass_utils import run_bass_kernel_spmd

    nc = get_nc()
    in_maps = make_in_maps(**inputs)
    res = run_bass_kernel_spmd(nc, in_maps, core_ids=list(range(8)))
    return assemble(res.results)
